# revision 19
# baseline (speedup 1.0000x reference)
"""Trainium2 Bass kernel for nn_PartRefinement.

Strategy (pure data parallel, 4 samples per core x 8 cores):

The reference's bilinear sampling is degenerate: with integer-cast weights,
only w11 = (x2-x1)*(y2-y1) in {0,1} survives, so projection is a single-pixel
gather masked by {0,1}.  We store images transposed ([S*S+1, C], zero row at
index S*S) in HBM and use dma_gather with index = masked ? x1*S+y1 : S*S.
The gather output [128, 8, C] is exactly the lhsT layout (points on
partitions) needed by the fc matmul (fc contracts over the point axis).

Everything after the single relu folds into two matmuls on host:
  f1  = c1_w_perm @ [img_fc ; point ; grid ; fc_b] + (c1_cg @ codes + c1_b)
  f1r = relu(W32 @ f1 + b32)          W32 = c3_w @ c2_w
  feat= WF @ f1r + bF                 WF folds w2d1/w2d2/w2d3/c4/c5/c6 chain
"""
import sys
from contextlib import ExitStack

import numpy as np
import ml_dtypes

BF = ml_dtypes.bfloat16

try:
    from concourse import bass, bacc, mybir, tile
except ImportError:  # fresh env without the axon site paths
    sys.path.insert(0, "/opt/trn_rl_repo")
    from concourse import bass, bacc, mybir, tile

from concourse.bass_utils import run_bass_kernel_spmd

F32 = mybir.dt.float32
BF16 = mybir.dt.bfloat16
I32 = mybir.dt.int32
I16 = mybir.dt.int16

B, N = 32, 1024
NCORES = 8
BPC = B // NCORES                     # samples per core
IMG_SIZES = [56, 28, 14, 7]
IMG_CH = [64, 128, 256, 512]
GCH = [128, 128, 256, 512]  # gather elem channels (img0 padded for 256B rule)
NH = 512                              # moving free dim (half of N)

# fc m-chunk -> (gather scale, column offset, width) in kernel channel order
# [img1(128) | img2(256) | img3(512) | img0(64)+point(3)+grid(2)+fcb(1)]
MCHUNKS = [
    (1, 0, 128), (2, 0, 128), (2, 128, 128),
    (3, 0, 128), (3, 128, 128), (3, 256, 128), (3, 384, 128),
    (0, 0, 64),
]

LAST_RESULTS = None                   # BassKernelResults of the last run
TRACE = False


def build_nc():
    nc = bacc.Bacc("TRN2", target_bir_lowering=False, debug=False)

    lvl = nc.declare_dram_parameter("lvl", [BPC, 3, N], F32, isOutput=False)
    lvl_bf = nc.declare_dram_parameter("lvl_bf", [BPC, 3, N], BF16, isOutput=False)
    its = [
        nc.declare_dram_parameter(
            f"it{i}", [BPC, IMG_SIZES[i] * IMG_SIZES[i] + 1, GCH[i]], BF16,
            isOutput=False)
        for i in range(4)
    ]
    fcwT = nc.declare_dram_parameter("fcwT", [1024, 1024], BF16, isOutput=False)
    c1wT = nc.declare_dram_parameter("c1wT", [966, 1024], BF16, isOutput=False)
    cgT = nc.declare_dram_parameter("cgT", [2048, 1024], F32, isOutput=False)
    codesT = nc.declare_dram_parameter("codesT", [2048, BPC], F32, isOutput=False)
    c1b = nc.declare_dram_parameter("c1b", [1024], F32, isOutput=False)
    w32T = nc.declare_dram_parameter("w32T", [1024, 64], BF16, isOutput=False)
    b32v = nc.declare_dram_parameter("b32v", [64], F32, isOutput=False)
    wfT = nc.declare_dram_parameter("wfT", [64, 6], BF16, isOutput=False)
    bfv = nc.declare_dram_parameter("bfv", [6], F32, isOutput=False)
    grid2 = nc.declare_dram_parameter("grid2", [2, N], BF16, isOutput=False)
    ctab = nc.declare_dram_parameter("ctab", [6, 1024], F32, isOutput=False)
    fcb = nc.declare_dram_parameter("fcb", [N], BF16, isOutput=False)
    feat = nc.declare_dram_parameter("feat", [BPC, 6, N], F32, isOutput=True)

    add, mult, subtract = (mybir.AluOpType.add, mybir.AluOpType.mult,
                           mybir.AluOpType.subtract)
    is_gt = mybir.AluOpType.is_gt
    amax, amin = mybir.AluOpType.max, mybir.AluOpType.min

    with tile.TileContext(nc) as tc, ExitStack() as es:
        def pool(name, bufs, space="SBUF"):
            return es.enter_context(
                tc.tile_pool(name=name, bufs=bufs, space=space))

        wp = pool("weights", 1)
        cgp = pool("cgpool", 3)
        scr = pool("scratch", 1)
        gp = pool("gather", 3)
        yep = pool("yext", 1)
        f1sb = pool("f1sb", 1)
        outp = pool("outsb", 2)
        psA = pool("psA", 2, "PSUM")
        psB = pool("psB", 2, "PSUM")
        psC = pool("psC", 1, "PSUM")
        psD = pool("psD", 1, "PSUM")
        psE = pool("psE", 2, "PSUM")

        # ---- coordinates & gather indices ---------------------------------
        # c16[p, c, b, i] = level0[b, c, i*16 + p]  (dma_gather wrap order),
        # then replicated to all 8 groups of 16 partitions.
        NW = BPC * 64                                # free width of coord math
        idx_ts = [scr.tile([128, BPC, 64], I16, name=f"idxs{s}", tag=f"idxs{s}")
                  for s in range(4)]

        with tc.tile_pool(name="idxtmp", bufs=1) as itp:
            c16 = itp.tile([16, 3, BPC, 64], F32, name="c16")
            for c in range(3):
                for b in range(BPC):
                    nc.sync.dma_start(
                        out=c16[:, c, b, :],
                        in_=lvl[b, c].rearrange("(i p) -> p i", p=16))
            coords = itp.tile([128, 3, BPC, 64], F32, name="coords")
            for g in range(8):
                nc.sync.dma_start(out=coords[16 * g:16 * (g + 1)], in_=c16[:])
            x0 = coords[:, 0].rearrange("p b i -> p (b i)")
            yy = coords[:, 1].rearrange("p b i -> p (b i)")
            zz = coords[:, 2].rearrange("p b i -> p (b i)")

            # per-column constants: rows CA, CB, CMAX, CLST, CS, CS2
            ctab_sb = itp.tile([1, 6 * 1024], F32, name="ctabsb")
            nc.sync.dma_start(
                out=ctab_sb[:],
                in_=ctab[:].rearrange("r w -> (r w)").unsqueeze(0))
            cb6 = []
            for r in range(6):
                ct_ = itp.tile([128, 1024], F32, name=f"cb{r}", tag=f"cb{r}")
                nc.gpsimd.partition_broadcast(
                    ct_[:], ctab_sb[:, r * 1024:(r + 1) * 1024])
                cb6.append(ct_)
            CA_t, CB_t, CMAX_t, CLST_t, CS_t, CS2_t = cb6

            def i_t(tag, wide=True, dt=F32):
                shape = [128, 4, NW] if wide else [128, NW]
                return itp.tile(shape, dt, name=tag, tag=tag)

            rz = i_t("rz", False)
            nc.vector.reciprocal(rz[:], zz)
            qh = i_t("qh", False)
            qw = i_t("qw", False)
            for qt, num in ((qh, yy), (qw, x0)):
                nc.vector.tensor_tensor(qt[:], num, rz[:], op=mult)  # q0
                # one Newton step: q = q0 + rz*(n - q0*z)
                t1 = i_t("nt1", False)
                nc.vector.tensor_tensor(t1[:], qt[:], zz, op=mult)
                nc.vector.tensor_tensor(t1[:], num, t1[:], op=subtract)
                nc.vector.tensor_tensor(t1[:], t1[:], rz[:], op=mult)
                nc.vector.tensor_tensor(qt[:], qt[:], t1[:], op=add)
            nc.vector.tensor_scalar_mul(qw[:], qw[:], -1.0)  # w uses -q

            xi = itp.tile([128, 2, NW], I32, name="xi", tag="xi")
            V = itp.tile([128, 2, NW], F32, name="V", tag="V")
            T = itp.tile([128, 2, NW], F32, name="T", tag="T")
            U = itp.tile([128, 2, NW], F32, name="U", tag="U")
            for s in (1, 2, 3, 0):       # match gather/fc consumption order
                ssl = slice(s * NW, (s + 1) * NW)
                nc.vector.tensor_copy(T[:, 0, :], qh[:])
                nc.vector.tensor_copy(T[:, 1, :], qw[:])
                for ct_, op_ in ((CA_t, mult), (CB_t, add)):
                    cc = ct_[:, ssl].unsqueeze(1).broadcast_to([128, 2, NW])
                    nc.vector.tensor_tensor(T[:], T[:], cc, op=op_)
                nc.vector.tensor_scalar_max(T[:], T[:], 0.0)
                cc = CMAX_t[:, ssl].unsqueeze(1).broadcast_to([128, 2, NW])
                nc.vector.tensor_tensor(T[:], T[:], cc, op=amin)     # xf
                nc.vector.tensor_copy(xi[:], T[:])                   # floor
                nc.vector.tensor_copy(U[:], xi[:])                   # xq
                nc.vector.tensor_tensor(V[:], U[:], T[:], op=is_gt)
                nc.vector.tensor_tensor(U[:], U[:], V[:], op=subtract)  # fl
                nc.vector.tensor_tensor(V[:], T[:], U[:], op=is_gt)  # hasfrac
                nc.vector.tensor_tensor(T[:], U[:], V[:], op=add)
                cc = CLST_t[:, ssl].unsqueeze(1).broadcast_to([128, 2, NW])
                nc.vector.tensor_tensor(T[:], T[:], cc, op=amin)     # x2
                nc.vector.tensor_tensor(T[:], T[:], U[:], op=subtract)  # mx
                # combine the two axes
                fl0, fl1 = U[:, 0, :], U[:, 1, :]
                mx0, mx1 = T[:, 0, :], T[:, 1, :]
                nc.vector.tensor_tensor(V[:, 0, :], mx0, mx1, op=mult)
                nc.vector.tensor_tensor(mx0, fl0, CS_t[:, ssl], op=mult)
                nc.vector.tensor_tensor(mx0, mx0, fl1, op=add)       # idx
                nc.vector.tensor_tensor(mx0, mx0, CS2_t[:, ssl], op=subtract)
                nc.vector.tensor_tensor(mx0, mx0, V[:, 0, :], op=mult)
                nc.vector.tensor_tensor(mx0, mx0, CS2_t[:, ssl], op=add)
                nc.vector.tensor_copy(
                    idx_ts[s].rearrange("p b i -> p (b i)"), mx0)
        idx_tiles = idx_ts

        # ---- gathers -------------------------------------------------------
        g_tiles = [None] * BPC

        def emit_gathers(b):
            tiles = [None] * 4
            for s in (1, 2, 3, 0):   # fc consumes m-chunks in this scale order
                C = GCH[s]
                gt_ = gp.tile([128, 8, C], BF16, name=f"g{s}_{b}", tag=f"g{s}")
                nc.gpsimd.dma_gather(
                    out_ap=gt_[:],
                    in_ap=its[s][b],
                    idxs_ap=idx_tiles[s][:, b, :],
                    num_idxs=N,
                    num_idxs_reg=N,
                    elem_size=C,
                    queue_num=0,
                )
                tiles[s] = gt_
            g_tiles[b] = tiles

        emit_gathers(0)
        emit_gathers(1)

        # ---- persistent weights -------------------------------------------
        fcw_t = wp.tile([128, 8, 1024], BF16, name="fcw")
        nc.sync.dma_start(out=fcw_t[:], in_=fcwT[:].rearrange("(k p) o -> p k o", p=128))
        c1wa = wp.tile([128, 7, 1024], BF16, name="c1wa")
        nc.sync.dma_start(out=c1wa[:],
                          in_=c1wT[0:896].rearrange("(k p) o -> p k o", p=128))
        c1wb = wp.tile([70, 1024], BF16, name="c1wb")
        nc.sync.dma_start(out=c1wb[:], in_=c1wT[896:966])
        w32_t = wp.tile([128, 8, 64], BF16, name="w32t")
        nc.sync.dma_start(out=w32_t[:], in_=w32T[:].rearrange("(k p) m -> p k m", p=128))
        wf_t = wp.tile([64, 6], BF16, name="wft")
        nc.sync.dma_start(out=wf_t[:], in_=wfT[:])
        b32_t = wp.tile([64, 1], F32, name="b32t")
        nc.sync.dma_start(out=b32_t[:], in_=b32v[:].unsqueeze(1))
        bf_t = wp.tile([6, 1], F32, name="bft")
        nc.sync.dma_start(out=bf_t[:], in_=bfv[:].unsqueeze(1))
        c1b_t = wp.tile([128, 8], F32, name="c1bt")
        nc.sync.dma_start(out=c1b_t[:], in_=c1b[:].rearrange("(m p) -> p m", p=128))
        codes_t = wp.tile([128, 16, BPC], F32, name="codest")
        nc.sync.dma_start(out=codes_t[:],
                          in_=codesT[:].rearrange("(k p) b -> p k b", p=128))


        # ---- CT = c1_cg @ [code; global_code] + c1_b  (all samples) -------
        # PSUM holds only one k-step (per-matmul start/stop); accumulate in
        # SBUF to avoid 8 interleaved accumulation groups in one bank.
        ct_sb = wp.tile([128, 8, BPC], F32, name="ctsb")
        nc.vector.memset(ct_sb[:], 0.0)
        for k in range(16):
            cg_t = cgp.tile([128, 1024], F32, name=f"cg{k}", tag="cg")
            nc.sync.dma_start(out=cg_t[:], in_=cgT[128 * k:128 * (k + 1)])
            ctp = psE.tile([128, 8, BPC], F32, name="ctp", tag="ctp")
            for m in range(8):
                nc.tensor.matmul(
                    ctp[:, m, :], (cg_t[:, 128 * m:128 * (m + 1)]),
                    (codes_t[:, k, :]), start=True, stop=True,
                )
            acc_v = ct_sb.rearrange("p m b -> p (m b)")
            nc.vector.tensor_tensor(acc_v, acc_v,
                                    ctp.rearrange("p m b -> p (m b)"), op=add)
        for m in range(8):
            nc.vector.tensor_scalar(out=ct_sb[:, m, :], in0=ct_sb[:, m, :],
                                    scalar1=c1b_t[:, m:m + 1], scalar2=None,
                                    op0=add)

        # ---- per-sample pipeline ------------------------------------------
        for b in range(BPC):
            g0, g1, g2, g3 = g_tiles[b]
            gmap = {0: g0, 1: g1, 2: g2, 3: g3}
            for oc in range(2):
                osl = slice(oc * NH, (oc + 1) * NH)
                # fc: Y[c, o] = sum_n GT[n, c] fcwT[n, o]
                ye = [yep.tile([128, NH], BF16, name=f"ye{k}", tag=f"ye{k}")
                      for k in range(7)]
                ye7 = yep.tile([70, NH], BF16, name="ye7", tag="ye7")
                for m, (s, c0, cw) in enumerate(MCHUNKS):
                    yp = psA.tile([cw, NH], F32, name="yp", tag="yp")
                    for k in range(8):
                        nc.tensor.matmul(
                            yp[:], (gmap[s][:, k, c0:c0 + cw]),
                            (fcw_t[:, k, osl]),
                            start=(k == 0), stop=(k == 7),
                        )
                    dst = ye[m][:] if m < 7 else ye7[0:64, :]
                    if m % 2 == 0:
                        nc.vector.tensor_copy(dst, yp[:])
                    else:
                        nc.scalar.copy(dst, yp[:])
                nc.sync.dma_start(out=ye7[64:67, :], in_=lvl_bf[b][:, osl])
                nc.sync.dma_start(out=ye7[67:69, :], in_=grid2[:, osl])
                nc.sync.dma_start(out=ye7[69:70, :], in_=fcb[osl].unsqueeze(0))

                # c1
                f1 = [f1sb.tile([128, NH], BF16, name=f"f1_{m}", tag=f"f1_{m}")
                      for m in range(8)]
                for m in range(8):
                    fp1 = psB.tile([128, NH], F32, name="f1p", tag="f1p")
                    for k in range(8):
                        if k < 7:
                            nc.tensor.matmul(
                                fp1[:], (c1wa[:, k, 128 * m:128 * (m + 1)]),
                                (ye[k][:]), start=(k == 0), stop=False)
                        else:
                            nc.tensor.matmul(
                                fp1[:], (c1wb[:, 128 * m:128 * (m + 1)]),
                                (ye7[:]), start=False, stop=True)
                    nc.vector.tensor_scalar(out=f1[m][:], in0=fp1[:],
                                            scalar1=ct_sb[:, m, b:b + 1],
                                            scalar2=None, op0=add)

                # W32 + relu
                rp = psC.tile([64, NH], F32, name="rp", tag="rp")
                for k in range(8):
                    nc.tensor.matmul(rp[:], (w32_t[:, k, :]), (f1[k][:]),
                                     start=(k == 0), stop=(k == 7))
                f1r = outp.tile([64, NH], BF16, name="f1r", tag="f1r")
                nc.scalar.activation(f1r[:], rp[:],
                                     mybir.ActivationFunctionType.Relu,
                                     bias=b32_t[:], scale=1.0)

                # WF
                fpp = psD.tile([6, NH], F32, name="fpp", tag="fpp")
                nc.tensor.matmul(fpp[:], (wf_t[:]), (f1r[:]))
                feat_sb = outp.tile([6, NH], F32, name="featsb", tag="featsb")
                nc.vector.tensor_scalar(out=feat_sb[:], in0=fpp[:],
                                        scalar1=bf_t[:], scalar2=None, op0=add)
                nc.sync.dma_start(out=feat[b][:, osl], in_=feat_sb[:])

            if b + 2 < BPC:
                emit_gathers(b + 2)

    nc.compile()
    return nc


def fold_weights(inp):
    f64 = np.float64
    g = lambda k: np.asarray(inp[k], f64)
    w2d1s = g('w2d1').sum(-1)
    W21 = g('w2d2') @ w2d1s
    b21 = g('w2d2') @ g('b2d1') + g('b2d2')
    BD3 = np.zeros((64, 128), f64)
    for u in range(2):
        BD3[np.arange(32) * 2 + u, u * 64:(u + 1) * 64] = g('w2d3')
    b3x = np.zeros(64, f64)
    b3x[0::2] = g('b2d3'); b3x[1::2] = g('b2d3')
    W321 = BD3 @ W21
    b321 = BD3 @ b21 + b3x
    W65 = g('c6_w') @ g('c5_w')
    b65 = g('c6_w') @ g('c5_b') + g('c6_b')
    W64 = W65 @ g('c4_w')
    b64 = W65 @ g('c4_b') + b65
    WF = W64 @ (np.eye(64) + W321)
    bF = W64 @ b321 + b64
    W32 = g('c3_w') @ g('c2_w')
    b32 = g('c3_w') @ g('c2_b') + g('c3_b')
    return (WF.astype(np.float32), bF.astype(np.float32),
            W32.astype(np.float32), b32.astype(np.float32))


def prep_in_maps(inputs):
    """Full inputs -> per-core in_maps for run_bass_kernel_spmd."""
    f32 = np.float32
    WF, bF, W32, b32 = fold_weights(inputs)
    c1_w = np.ascontiguousarray(np.asarray(inputs['c1_w'], f32))
    ci = c1_w[:, 2053:3013]
    # kernel channel order: img1, img2, img3, img0, point, grid, fc_b-row
    c1wT = np.concatenate([
        ci[:, 64:192].T, ci[:, 192:448].T, ci[:, 448:960].T, ci[:, 0:64].T,
        c1_w[:, 2:5].T, c1_w[:, 0:2].T, ci.sum(axis=1)[None, :],
    ], axis=0).astype(BF)                                    # [966, 1024]
    cgT = np.ascontiguousarray(c1_w[:, 5:2053].T)            # [2048, 1024]
    fcwT = np.ascontiguousarray(np.asarray(inputs['fc_w'], f32).T.astype(BF))
    w32T = np.ascontiguousarray(W32.T.astype(BF))
    wfT = np.ascontiguousarray(WF.T.astype(BF))
    grid2 = np.zeros((2, N), BF)
    grid2[0, 0::2] = BF(-0.2)
    grid2[0, 1::2] = BF(0.2)
    grid2[1, :] = BF(-0.2)

    ctab = np.zeros((6, 1024), f32)
    for s, S in enumerate(IMG_SIZES):
        sl = slice(s * 256, (s + 1) * 256)
        ctab[0, sl] = f32(248.0) * f32(S / 224.0)
        ctab[1, sl] = f32(111.5) * f32(S / 224.0)
        ctab[2, sl] = f32(223.0) * f32(S / 224.0)
        ctab[3, sl] = f32(S - 1)
        ctab[4, sl] = f32(S)
        ctab[5, sl] = f32(S * S)

    codes = np.concatenate([np.asarray(inputs['code'], f32),
                            np.asarray(inputs['global_code'], f32)],
                           axis=1)                           # [B, 2048]
    level0 = np.asarray(inputs['level0'], f32)

    imgsT = []
    for i, S in enumerate(IMG_SIZES):
        img = np.asarray(inputs[f'img{i}'], f32)
        C = img.shape[1]
        t = img.reshape(B, C, S * S).transpose(0, 2, 1).astype(BF)
        full = np.zeros((B, S * S + 1, GCH[i]), BF)          # zero row + pad
        full[:, :S * S, :C] = t
        imgsT.append(full)

    shared = dict(fcwT=fcwT, c1wT=c1wT, cgT=cgT,
                  c1b=np.ascontiguousarray(np.asarray(inputs['c1_b'], f32)),
                  w32T=w32T, b32v=b32, wfT=wfT, bfv=bF,
                  grid2=grid2, ctab=ctab,
                  fcb=np.ascontiguousarray(
                      np.asarray(inputs['fc_b'], f32).astype(BF)))
    in_maps = []
    for c in range(NCORES):
        sl = slice(c * BPC, (c + 1) * BPC)
        m = dict(shared)
        m['lvl'] = np.ascontiguousarray(level0[sl])
        m['lvl_bf'] = np.ascontiguousarray(level0[sl].astype(BF))
        m['codesT'] = np.ascontiguousarray(codes[sl].T)
        for i in range(4):
            m[f'it{i}'] = imgsT[i][sl]
        in_maps.append(m)
    return in_maps


def assemble(results):
    out = np.zeros((B, 2 * N, 3), np.float32)
    for c in range(NCORES):
        featc = results[c]['feat']                   # [BPC, 6, N]
        for j in range(3):
            out[c * BPC:(c + 1) * BPC, :N, j] = featc[:, 2 * j, :]
            out[c * BPC:(c + 1) * BPC, N:, j] = featc[:, 2 * j + 1, :]
    return out


_NC_CACHE = None


def get_nc():
    global _NC_CACHE
    if _NC_CACHE is None:
        _NC_CACHE = build_nc()
    return _NC_CACHE


def kernel(**inputs):
    global LAST_RESULTS
    nc = get_nc()
    in_maps = prep_in_maps(inputs)
    res = run_bass_kernel_spmd(nc, in_maps, core_ids=list(range(NCORES)),
                               trace=TRACE)
    LAST_RESULTS = res
    return assemble(res.results)


# revision 20
# speedup vs baseline: 1.1371x; 1.1371x over previous
"""Trainium2 Bass kernel for nn_PartRefinement.

Strategy (pure data parallel, 4 samples per core x 8 cores):

The reference's bilinear sampling is degenerate: with integer-cast weights,
only w11 = (x2-x1)*(y2-y1) in {0,1} survives, so projection is a single-pixel
gather masked by {0,1}.  We store images transposed ([S*S+1, C], zero row at
index S*S) in HBM and use dma_gather with index = masked ? x1*S+y1 : S*S.
The gather output [128, 8, C] is exactly the lhsT layout (points on
partitions) needed by the fc matmul (fc contracts over the point axis).

Everything after the single relu folds into two matmuls on host:
  f1  = c1_w_perm @ [img_fc ; point ; grid ; fc_b] + (c1_cg @ codes + c1_b)
  f1r = relu(W32 @ f1 + b32)          W32 = c3_w @ c2_w
  feat= WF @ f1r + bF                 WF folds w2d1/w2d2/w2d3/c4/c5/c6 chain
"""
import sys
from contextlib import ExitStack

import numpy as np
import ml_dtypes

BF = ml_dtypes.bfloat16

try:
    from concourse import bass, bacc, mybir, tile
except ImportError:  # fresh env without the axon site paths
    sys.path.insert(0, "/opt/trn_rl_repo")
    from concourse import bass, bacc, mybir, tile

from concourse.bass_utils import run_bass_kernel_spmd

F32 = mybir.dt.float32
BF16 = mybir.dt.bfloat16
I32 = mybir.dt.int32
I16 = mybir.dt.int16

B, N = 32, 1024
NCORES = 8
BPC = B // NCORES                     # samples per core
IMG_SIZES = [56, 28, 14, 7]
IMG_CH = [64, 128, 256, 512]
GCH = [128, 128, 256, 512]  # gather elem channels (img0 padded for 256B rule)
NH = 512                              # moving free dim (half of N)

# fc m-chunk -> (gather scale, column offset, width) in kernel channel order
# [img1(128) | img2(256) | img3(512) | img0(64)+point(3)+grid(2)+fcb(1)]
MCHUNKS = [
    (1, 0, 128), (2, 0, 128), (2, 128, 128),
    (3, 0, 128), (3, 128, 128), (3, 256, 128), (3, 384, 128),
    (0, 0, 64),
]

LAST_RESULTS = None                   # BassKernelResults of the last run
TRACE = False


def build_nc():
    nc = bacc.Bacc("TRN2", target_bir_lowering=False, debug=False)

    lvl = nc.declare_dram_parameter("lvl", [BPC, 3, N], F32, isOutput=False)
    lvl_bf = nc.declare_dram_parameter("lvl_bf", [BPC, 3, N], BF16, isOutput=False)
    its = [
        nc.declare_dram_parameter(
            f"it{i}", [BPC, IMG_SIZES[i] * IMG_SIZES[i] + 1, GCH[i]], BF16,
            isOutput=False)
        for i in range(4)
    ]
    fcwT = nc.declare_dram_parameter("fcwT", [1024, 1024], BF16, isOutput=False)
    c1wT = nc.declare_dram_parameter("c1wT", [966, 1024], BF16, isOutput=False)
    cgT = nc.declare_dram_parameter("cgT", [2048, 1024], F32, isOutput=False)
    codesT = nc.declare_dram_parameter("codesT", [2048, BPC], F32, isOutput=False)
    c1b = nc.declare_dram_parameter("c1b", [1024], F32, isOutput=False)
    w32T = nc.declare_dram_parameter("w32T", [1024, 64], BF16, isOutput=False)
    b32v = nc.declare_dram_parameter("b32v", [64], F32, isOutput=False)
    wfT = nc.declare_dram_parameter("wfT", [64, 6], BF16, isOutput=False)
    bfv = nc.declare_dram_parameter("bfv", [6], F32, isOutput=False)
    grid2 = nc.declare_dram_parameter("grid2", [2, N], BF16, isOutput=False)
    ctab = nc.declare_dram_parameter("ctab", [6, 1024], F32, isOutput=False)
    fcb = nc.declare_dram_parameter("fcb", [N], BF16, isOutput=False)
    feat = nc.declare_dram_parameter("feat", [BPC, 6, N], F32, isOutput=True)

    add, mult, subtract = (mybir.AluOpType.add, mybir.AluOpType.mult,
                           mybir.AluOpType.subtract)
    is_gt = mybir.AluOpType.is_gt
    amax, amin = mybir.AluOpType.max, mybir.AluOpType.min

    with tile.TileContext(nc) as tc, ExitStack() as es:
        def pool(name, bufs, space="SBUF"):
            return es.enter_context(
                tc.tile_pool(name=name, bufs=bufs, space=space))

        wp = pool("weights", 1)
        cgp = pool("cgpool", 3)
        scr = pool("scratch", 1)
        gp = pool("gather", 3)
        yep = pool("yext", 1)
        f1sb = pool("f1sb", 1)
        outp = pool("outsb", 2)
        psA = pool("psA", 2, "PSUM")
        psB = pool("psB", 2, "PSUM")
        psC = pool("psC", 1, "PSUM")
        psD = pool("psD", 1, "PSUM")
        psE = pool("psE", 2, "PSUM")

        # ---- coordinates & gather indices ---------------------------------
        # c16[p, c, b, i] = level0[b, c, i*16 + p]  (dma_gather wrap order),
        # then replicated to all 8 groups of 16 partitions.
        NW = BPC * 64                                # free width of coord math
        idx_ts = [scr.tile([128, BPC, 64], I16, name=f"idxs{s}", tag=f"idxs{s}")
                  for s in range(4)]

        with tc.tile_pool(name="idxtmp", bufs=1) as itp:
            c16 = itp.tile([16, 3, BPC, 64], F32, name="c16")
            for c in range(3):
                for b in range(BPC):
                    nc.sync.dma_start(
                        out=c16[:, c, b, :],
                        in_=lvl[b, c].rearrange("(i p) -> p i", p=16))
            coords = itp.tile([128, 3, BPC, 64], F32, name="coords")
            for g in range(8):
                nc.sync.dma_start(out=coords[16 * g:16 * (g + 1)], in_=c16[:])
            x0 = coords[:, 0].rearrange("p b i -> p (b i)")
            yy = coords[:, 1].rearrange("p b i -> p (b i)")
            zz = coords[:, 2].rearrange("p b i -> p (b i)")

            # per-column constants: rows CA, CB, CMAX, CLST, CS, CS2
            ctab_sb = itp.tile([1, 6 * 1024], F32, name="ctabsb")
            nc.sync.dma_start(
                out=ctab_sb[:],
                in_=ctab[:].rearrange("r w -> (r w)").unsqueeze(0))
            cb6 = []
            for r in range(6):
                ct_ = itp.tile([128, 1024], F32, name=f"cb{r}", tag=f"cb{r}")
                nc.gpsimd.partition_broadcast(
                    ct_[:], ctab_sb[:, r * 1024:(r + 1) * 1024])
                cb6.append(ct_)
            CA_t, CB_t, CMAX_t, CLST_t, CS_t, CS2_t = cb6

            def i_t(tag, wide=True, dt=F32):
                shape = [128, 4, NW] if wide else [128, NW]
                return itp.tile(shape, dt, name=tag, tag=tag)

            rz = i_t("rz", False)
            nc.vector.reciprocal(rz[:], zz)
            qh = i_t("qh", False)
            qw = i_t("qw", False)
            for qt, num in ((qh, yy), (qw, x0)):
                nc.vector.tensor_tensor(qt[:], num, rz[:], op=mult)  # q0
                # one Newton step: q = q0 + rz*(n - q0*z)
                t1 = i_t("nt1", False)
                nc.vector.tensor_tensor(t1[:], qt[:], zz, op=mult)
                nc.vector.tensor_tensor(t1[:], num, t1[:], op=subtract)
                nc.vector.tensor_tensor(t1[:], t1[:], rz[:], op=mult)
                nc.vector.tensor_tensor(qt[:], qt[:], t1[:], op=add)
            nc.vector.tensor_scalar_mul(qw[:], qw[:], -1.0)  # w uses -q

            xi = itp.tile([128, 2, 4, NW], I32, name="xi", tag="xi")
            V = itp.tile([128, 2, 4, NW], F32, name="V", tag="V")
            T = itp.tile([128, 2, 4, NW], F32, name="T", tag="T")
            U = itp.tile([128, 2, 4, NW], F32, name="U", tag="U")
            Tf = T.rearrange("p a s w -> p (a s w)")
            Uf = U.rearrange("p a s w -> p (a s w)")
            Vf = V.rearrange("p a s w -> p (a s w)")
            xif = xi.rearrange("p a s w -> p (a s w)")
            CAb = CA_t[:].unsqueeze(1).broadcast_to([128, 2, 1024])
            CBb = CB_t[:].unsqueeze(1).broadcast_to([128, 2, 1024])
            CMb = CMAX_t[:].unsqueeze(1).broadcast_to([128, 2, 1024])
            CLb = CLST_t[:].unsqueeze(1).broadcast_to([128, 2, 1024])
            for a, qt in ((0, qh), (1, qw)):
                for s in range(4):
                    nc.vector.tensor_copy(T[:, a, s, :], qt[:])
            nc.vector.tensor_tensor(T[:], T[:], CAb, op=mult)
            nc.vector.tensor_tensor(T[:], T[:], CBb, op=add)
            nc.vector.tensor_scalar_max(Tf, Tf, 0.0)
            nc.vector.tensor_tensor(T[:], T[:], CMb, op=amin)        # xf
            nc.vector.tensor_copy(xif, Tf)                           # floor
            nc.vector.tensor_copy(Uf, xif)                           # xq
            nc.vector.tensor_tensor(Vf, Uf, Tf, op=is_gt)
            nc.vector.tensor_tensor(Uf, Uf, Vf, op=subtract)         # fl
            nc.vector.tensor_tensor(Vf, Tf, Uf, op=is_gt)            # hasfrac
            nc.vector.tensor_tensor(Tf, Uf, Vf, op=add)
            nc.vector.tensor_tensor(T[:], T[:], CLb, op=amin)        # x2
            nc.vector.tensor_tensor(Tf, Tf, Uf, op=subtract)         # mx
            fl0 = U[:, 0].rearrange("p s w -> p (s w)")
            fl1 = U[:, 1].rearrange("p s w -> p (s w)")
            mx0 = T[:, 0].rearrange("p s w -> p (s w)")
            mx1 = T[:, 1].rearrange("p s w -> p (s w)")
            Vc = V[:, 0].rearrange("p s w -> p (s w)")
            nc.vector.tensor_tensor(Vc, mx0, mx1, op=mult)           # mask
            nc.vector.tensor_tensor(mx0, fl0, CS_t[:], op=mult)
            nc.vector.tensor_tensor(mx0, mx0, fl1, op=add)           # idx
            nc.vector.tensor_tensor(mx0, mx0, CS2_t[:], op=subtract)
            nc.vector.tensor_tensor(mx0, mx0, Vc, op=mult)
            nc.vector.tensor_tensor(mx0, mx0, CS2_t[:], op=add)
            for s in (1, 2, 3, 0):
                nc.vector.tensor_copy(
                    idx_ts[s].rearrange("p b i -> p (b i)"),
                    T[:, 0, s, :].rearrange("p b i -> p (b i)") if False
                    else mx0[:, s * NW:(s + 1) * NW])
        idx_tiles = idx_ts

        # ---- gathers -------------------------------------------------------
        g_tiles = [None] * BPC

        def emit_gathers(b):
            tiles = [None] * 4
            for s in (1, 2, 3, 0):   # fc consumes m-chunks in this scale order
                C = GCH[s]
                gt_ = gp.tile([128, 8, C], BF16, name=f"g{s}_{b}", tag=f"g{s}")
                nc.gpsimd.dma_gather(
                    out_ap=gt_[:],
                    in_ap=its[s][b],
                    idxs_ap=idx_tiles[s][:, b, :],
                    num_idxs=N,
                    num_idxs_reg=N,
                    elem_size=C,
                    queue_num=0,
                )
                tiles[s] = gt_
            g_tiles[b] = tiles

        emit_gathers(0)
        emit_gathers(1)

        # ---- persistent weights -------------------------------------------
        fcw_t = wp.tile([128, 8, 1024], BF16, name="fcw")
        nc.sync.dma_start(out=fcw_t[:], in_=fcwT[:].rearrange("(k p) o -> p k o", p=128))
        c1wa = wp.tile([128, 7, 1024], BF16, name="c1wa")
        nc.sync.dma_start(out=c1wa[:],
                          in_=c1wT[0:896].rearrange("(k p) o -> p k o", p=128))
        c1wb = wp.tile([70, 1024], BF16, name="c1wb")
        nc.sync.dma_start(out=c1wb[:], in_=c1wT[896:966])
        w32_t = wp.tile([128, 8, 64], BF16, name="w32t")
        nc.sync.dma_start(out=w32_t[:], in_=w32T[:].rearrange("(k p) m -> p k m", p=128))
        wf_t = wp.tile([64, 6], BF16, name="wft")
        nc.sync.dma_start(out=wf_t[:], in_=wfT[:])
        b32_t = wp.tile([64, 1], F32, name="b32t")
        nc.sync.dma_start(out=b32_t[:], in_=b32v[:].unsqueeze(1))
        bf_t = wp.tile([6, 1], F32, name="bft")
        nc.sync.dma_start(out=bf_t[:], in_=bfv[:].unsqueeze(1))
        c1b_t = wp.tile([128, 8], F32, name="c1bt")
        nc.sync.dma_start(out=c1b_t[:], in_=c1b[:].rearrange("(m p) -> p m", p=128))
        codes_t = wp.tile([128, 16, BPC], F32, name="codest")
        nc.sync.dma_start(out=codes_t[:],
                          in_=codesT[:].rearrange("(k p) b -> p k b", p=128))


        # ---- CT = c1_cg @ [code; global_code] + c1_b  (all samples) -------
        # PSUM holds only one k-step (per-matmul start/stop); accumulate in
        # SBUF to avoid 8 interleaved accumulation groups in one bank.
        ct_sb = wp.tile([128, 8, BPC], F32, name="ctsb")
        nc.vector.memset(ct_sb[:], 0.0)
        for k in range(16):
            cg_t = cgp.tile([128, 1024], F32, name=f"cg{k}", tag="cg")
            nc.sync.dma_start(out=cg_t[:], in_=cgT[128 * k:128 * (k + 1)])
            ctp = psE.tile([128, 8, BPC], F32, name="ctp", tag="ctp")
            for m in range(8):
                nc.tensor.matmul(
                    ctp[:, m, :], (cg_t[:, 128 * m:128 * (m + 1)]),
                    (codes_t[:, k, :]), start=True, stop=True,
                )
            acc_v = ct_sb.rearrange("p m b -> p (m b)")
            nc.vector.tensor_tensor(acc_v, acc_v,
                                    ctp.rearrange("p m b -> p (m b)"), op=add)
        for m in range(8):
            nc.vector.tensor_scalar(out=ct_sb[:, m, :], in0=ct_sb[:, m, :],
                                    scalar1=c1b_t[:, m:m + 1], scalar2=None,
                                    op0=add)

        # ---- per-sample pipeline ------------------------------------------
        for b in range(BPC):
            g0, g1, g2, g3 = g_tiles[b]
            gmap = {0: g0, 1: g1, 2: g2, 3: g3}
            for oc in range(2):
                osl = slice(oc * NH, (oc + 1) * NH)
                # fc: Y[c, o] = sum_n GT[n, c] fcwT[n, o]
                ye = [yep.tile([128, NH], BF16, name=f"ye{k}", tag=f"ye{k}")
                      for k in range(7)]
                ye7 = yep.tile([70, NH], BF16, name="ye7", tag="ye7")
                for m, (s, c0, cw) in enumerate(MCHUNKS):
                    yp = psA.tile([cw, NH], F32, name="yp", tag="yp")
                    for k in range(8):
                        nc.tensor.matmul(
                            yp[:], (gmap[s][:, k, c0:c0 + cw]),
                            (fcw_t[:, k, osl]),
                            start=(k == 0), stop=(k == 7),
                        )
                    dst = ye[m][:] if m < 7 else ye7[0:64, :]
                    nc.vector.tensor_copy(dst, yp[:])
                nc.sync.dma_start(out=ye7[64:67, :], in_=lvl_bf[b][:, osl])
                nc.sync.dma_start(out=ye7[67:69, :], in_=grid2[:, osl])
                nc.sync.dma_start(out=ye7[69:70, :], in_=fcb[osl].unsqueeze(0))

                # c1
                f1 = [f1sb.tile([128, NH], BF16, name=f"f1_{m}", tag=f"f1_{m}")
                      for m in range(8)]
                for m in range(8):
                    fp1 = psB.tile([128, NH], F32, name="f1p", tag="f1p")
                    for k in range(8):
                        if k < 7:
                            nc.tensor.matmul(
                                fp1[:], (c1wa[:, k, 128 * m:128 * (m + 1)]),
                                (ye[k][:]), start=(k == 0), stop=False)
                        else:
                            nc.tensor.matmul(
                                fp1[:], (c1wb[:, 128 * m:128 * (m + 1)]),
                                (ye7[:]), start=False, stop=True)
                    nc.vector.tensor_scalar(out=f1[m][:], in0=fp1[:],
                                            scalar1=ct_sb[:, m, b:b + 1],
                                            scalar2=None, op0=add)

                # W32 + relu
                rp = psC.tile([64, NH], F32, name="rp", tag="rp")
                for k in range(8):
                    nc.tensor.matmul(rp[:], (w32_t[:, k, :]), (f1[k][:]),
                                     start=(k == 0), stop=(k == 7))
                f1r = outp.tile([64, NH], BF16, name="f1r", tag="f1r")
                nc.scalar.activation(f1r[:], rp[:],
                                     mybir.ActivationFunctionType.Relu,
                                     bias=b32_t[:], scale=1.0)

                # WF
                fpp = psD.tile([6, NH], F32, name="fpp", tag="fpp")
                nc.tensor.matmul(fpp[:], (wf_t[:]), (f1r[:]))
                feat_sb = outp.tile([6, NH], F32, name="featsb", tag="featsb")
                nc.vector.tensor_scalar(out=feat_sb[:], in0=fpp[:],
                                        scalar1=bf_t[:], scalar2=None, op0=add)
                nc.sync.dma_start(out=feat[b][:, osl], in_=feat_sb[:])

            if b + 2 < BPC:
                emit_gathers(b + 2)

    nc.compile()
    return nc


def fold_weights(inp):
    f64 = np.float64
    g = lambda k: np.asarray(inp[k], f64)
    w2d1s = g('w2d1').sum(-1)
    W21 = g('w2d2') @ w2d1s
    b21 = g('w2d2') @ g('b2d1') + g('b2d2')
    BD3 = np.zeros((64, 128), f64)
    for u in range(2):
        BD3[np.arange(32) * 2 + u, u * 64:(u + 1) * 64] = g('w2d3')
    b3x = np.zeros(64, f64)
    b3x[0::2] = g('b2d3'); b3x[1::2] = g('b2d3')
    W321 = BD3 @ W21
    b321 = BD3 @ b21 + b3x
    W65 = g('c6_w') @ g('c5_w')
    b65 = g('c6_w') @ g('c5_b') + g('c6_b')
    W64 = W65 @ g('c4_w')
    b64 = W65 @ g('c4_b') + b65
    WF = W64 @ (np.eye(64) + W321)
    bF = W64 @ b321 + b64
    W32 = g('c3_w') @ g('c2_w')
    b32 = g('c3_w') @ g('c2_b') + g('c3_b')
    return (WF.astype(np.float32), bF.astype(np.float32),
            W32.astype(np.float32), b32.astype(np.float32))


def prep_in_maps(inputs):
    """Full inputs -> per-core in_maps for run_bass_kernel_spmd."""
    f32 = np.float32
    WF, bF, W32, b32 = fold_weights(inputs)
    c1_w = np.ascontiguousarray(np.asarray(inputs['c1_w'], f32))
    ci = c1_w[:, 2053:3013]
    # kernel channel order: img1, img2, img3, img0, point, grid, fc_b-row
    c1wT = np.concatenate([
        ci[:, 64:192].T, ci[:, 192:448].T, ci[:, 448:960].T, ci[:, 0:64].T,
        c1_w[:, 2:5].T, c1_w[:, 0:2].T, ci.sum(axis=1)[None, :],
    ], axis=0).astype(BF)                                    # [966, 1024]
    cgT = np.ascontiguousarray(c1_w[:, 5:2053].T)            # [2048, 1024]
    fcwT = np.ascontiguousarray(np.asarray(inputs['fc_w'], f32).T.astype(BF))
    w32T = np.ascontiguousarray(W32.T.astype(BF))
    wfT = np.ascontiguousarray(WF.T.astype(BF))
    grid2 = np.zeros((2, N), BF)
    grid2[0, 0::2] = BF(-0.2)
    grid2[0, 1::2] = BF(0.2)
    grid2[1, :] = BF(-0.2)

    ctab = np.zeros((6, 1024), f32)
    for s, S in enumerate(IMG_SIZES):
        sl = slice(s * 256, (s + 1) * 256)
        ctab[0, sl] = f32(248.0) * f32(S / 224.0)
        ctab[1, sl] = f32(111.5) * f32(S / 224.0)
        ctab[2, sl] = f32(223.0) * f32(S / 224.0)
        ctab[3, sl] = f32(S - 1)
        ctab[4, sl] = f32(S)
        ctab[5, sl] = f32(S * S)

    codes = np.concatenate([np.asarray(inputs['code'], f32),
                            np.asarray(inputs['global_code'], f32)],
                           axis=1)                           # [B, 2048]
    level0 = np.asarray(inputs['level0'], f32)

    imgsT = []
    for i, S in enumerate(IMG_SIZES):
        img = np.asarray(inputs[f'img{i}'], f32)
        C = img.shape[1]
        t = img.reshape(B, C, S * S).transpose(0, 2, 1).astype(BF)
        full = np.zeros((B, S * S + 1, GCH[i]), BF)          # zero row + pad
        full[:, :S * S, :C] = t
        imgsT.append(full)

    shared = dict(fcwT=fcwT, c1wT=c1wT, cgT=cgT,
                  c1b=np.ascontiguousarray(np.asarray(inputs['c1_b'], f32)),
                  w32T=w32T, b32v=b32, wfT=wfT, bfv=bF,
                  grid2=grid2, ctab=ctab,
                  fcb=np.ascontiguousarray(
                      np.asarray(inputs['fc_b'], f32).astype(BF)))
    in_maps = []
    for c in range(NCORES):
        sl = slice(c * BPC, (c + 1) * BPC)
        m = dict(shared)
        m['lvl'] = np.ascontiguousarray(level0[sl])
        m['lvl_bf'] = np.ascontiguousarray(level0[sl].astype(BF))
        m['codesT'] = np.ascontiguousarray(codes[sl].T)
        for i in range(4):
            m[f'it{i}'] = imgsT[i][sl]
        in_maps.append(m)
    return in_maps


def assemble(results):
    out = np.zeros((B, 2 * N, 3), np.float32)
    for c in range(NCORES):
        featc = results[c]['feat']                   # [BPC, 6, N]
        for j in range(3):
            out[c * BPC:(c + 1) * BPC, :N, j] = featc[:, 2 * j, :]
            out[c * BPC:(c + 1) * BPC, N:, j] = featc[:, 2 * j + 1, :]
    return out


_NC_CACHE = None


def get_nc():
    global _NC_CACHE
    if _NC_CACHE is None:
        _NC_CACHE = build_nc()
    return _NC_CACHE


def kernel(**inputs):
    global LAST_RESULTS
    nc = get_nc()
    in_maps = prep_in_maps(inputs)
    res = run_bass_kernel_spmd(nc, in_maps, core_ids=list(range(NCORES)),
                               trace=TRACE)
    LAST_RESULTS = res
    return assemble(res.results)


# revision 21
# speedup vs baseline: 1.1464x; 1.0082x over previous
"""Trainium2 Bass kernel for nn_PartRefinement.

Strategy (pure data parallel, 4 samples per core x 8 cores):

The reference's bilinear sampling is degenerate: with integer-cast weights,
only w11 = (x2-x1)*(y2-y1) in {0,1} survives, so projection is a single-pixel
gather masked by {0,1}.  We store images transposed ([S*S+1, C], zero row at
index S*S) in HBM and use dma_gather with index = masked ? x1*S+y1 : S*S.
The gather output [128, 8, C] is exactly the lhsT layout (points on
partitions) needed by the fc matmul (fc contracts over the point axis).

Everything after the single relu folds into two matmuls on host:
  f1  = c1_w_perm @ [img_fc ; point ; grid ; fc_b] + (c1_cg @ codes + c1_b)
  f1r = relu(W32 @ f1 + b32)          W32 = c3_w @ c2_w
  feat= WF @ f1r + bF                 WF folds w2d1/w2d2/w2d3/c4/c5/c6 chain
"""
import sys
from contextlib import ExitStack

import numpy as np
import ml_dtypes

BF = ml_dtypes.bfloat16

try:
    from concourse import bass, bacc, mybir, tile
except ImportError:  # fresh env without the axon site paths
    sys.path.insert(0, "/opt/trn_rl_repo")
    from concourse import bass, bacc, mybir, tile

from concourse.bass_utils import run_bass_kernel_spmd

F32 = mybir.dt.float32
BF16 = mybir.dt.bfloat16
I32 = mybir.dt.int32
I16 = mybir.dt.int16

B, N = 32, 1024
NCORES = 8
BPC = B // NCORES                     # samples per core
IMG_SIZES = [56, 28, 14, 7]
IMG_CH = [64, 128, 256, 512]
GCH = [128, 128, 256, 512]  # gather elem channels (img0 padded for 256B rule)
NH = 512                              # moving free dim (half of N)

# fc m-chunk -> (gather scale, column offset, width) in kernel channel order
# [img1(128) | img2(256) | img3(512) | img0(64)+point(3)+grid(2)+fcb(1)]
MCHUNKS = [
    (1, 0, 128), (2, 0, 128), (2, 128, 128),
    (3, 0, 128), (3, 128, 128), (3, 256, 128), (3, 384, 128),
    (0, 0, 64),
]

LAST_RESULTS = None                   # BassKernelResults of the last run
TRACE = False


def build_nc():
    nc = bacc.Bacc("TRN2", target_bir_lowering=False, debug=False)

    lvl = nc.declare_dram_parameter("lvl", [BPC, 3, N], F32, isOutput=False)
    lvl_bf = nc.declare_dram_parameter("lvl_bf", [BPC, 3, N], BF16, isOutput=False)
    its = [
        nc.declare_dram_parameter(
            f"it{i}", [BPC, IMG_SIZES[i] * IMG_SIZES[i] + 1, GCH[i]], BF16,
            isOutput=False)
        for i in range(4)
    ]
    fcwT = nc.declare_dram_parameter("fcwT", [1024, 1024], BF16, isOutput=False)
    c1wT = nc.declare_dram_parameter("c1wT", [966, 1024], BF16, isOutput=False)
    cgT = nc.declare_dram_parameter("cgT", [2048, 1024], F32, isOutput=False)
    codesT = nc.declare_dram_parameter("codesT", [2048, BPC], F32, isOutput=False)
    c1b = nc.declare_dram_parameter("c1b", [1024], F32, isOutput=False)
    w32T = nc.declare_dram_parameter("w32T", [1024, 64], BF16, isOutput=False)
    b32v = nc.declare_dram_parameter("b32v", [64], F32, isOutput=False)
    wfT = nc.declare_dram_parameter("wfT", [64, 6], BF16, isOutput=False)
    bfv = nc.declare_dram_parameter("bfv", [6], F32, isOutput=False)
    grid2 = nc.declare_dram_parameter("grid2", [2, N], BF16, isOutput=False)
    ctab = nc.declare_dram_parameter("ctab", [6, 1024], F32, isOutput=False)
    fcb = nc.declare_dram_parameter("fcb", [N], BF16, isOutput=False)
    feat = nc.declare_dram_parameter("feat", [BPC, 6, N], F32, isOutput=True)

    add, mult, subtract = (mybir.AluOpType.add, mybir.AluOpType.mult,
                           mybir.AluOpType.subtract)
    is_gt = mybir.AluOpType.is_gt
    amax, amin = mybir.AluOpType.max, mybir.AluOpType.min

    with tile.TileContext(nc) as tc, ExitStack() as es:
        def pool(name, bufs, space="SBUF"):
            return es.enter_context(
                tc.tile_pool(name=name, bufs=bufs, space=space))

        wp = pool("weights", 1)
        cgp = pool("cgpool", 3)
        scr = pool("scratch", 1)
        gp = pool("gather", 3)
        yep = pool("yext", 1)
        f1sb = pool("f1sb", 1)
        outp = pool("outsb", 2)
        psA = pool("psA", 2, "PSUM")
        psB = pool("psB", 2, "PSUM")
        psC = pool("psC", 1, "PSUM")
        psD = pool("psD", 1, "PSUM")
        psE = pool("psE", 2, "PSUM")

        # ---- coordinates & gather indices ---------------------------------
        # c16[p, c, b, i] = level0[b, c, i*16 + p]  (dma_gather wrap order),
        # then replicated to all 8 groups of 16 partitions.
        NW = BPC * 64                                # free width of coord math
        idx_ts = [scr.tile([128, BPC, 64], I16, name=f"idxs{s}", tag=f"idxs{s}")
                  for s in range(4)]

        with tc.tile_pool(name="idxtmp", bufs=1) as itp:
            c16 = itp.tile([16, 3, BPC, 64], F32, name="c16")
            for b in range(BPC):
                for c in range(3):
                    nc.sync.dma_start(
                        out=c16[:, c, b, :],
                        in_=lvl[b, c].rearrange("(i p) -> p i", p=16))
            coords = itp.tile([128, 3, BPC, 64], F32, name="coords")
            for g in range(8):
                nc.sync.dma_start(out=coords[16 * g:16 * (g + 1)], in_=c16[:])
            x0 = coords[:, 0].rearrange("p b i -> p (b i)")
            yy = coords[:, 1].rearrange("p b i -> p (b i)")
            zz = coords[:, 2].rearrange("p b i -> p (b i)")

            # per-column constants: rows CA, CB, CMAX, CLST, CS, CS2
            ctab_sb = itp.tile([1, 6 * 1024], F32, name="ctabsb")
            nc.sync.dma_start(
                out=ctab_sb[:],
                in_=ctab[:].rearrange("r w -> (r w)").unsqueeze(0))
            cb6 = []
            for r in range(6):
                ct_ = itp.tile([128, 1024], F32, name=f"cb{r}", tag=f"cb{r}")
                nc.gpsimd.partition_broadcast(
                    ct_[:], ctab_sb[:, r * 1024:(r + 1) * 1024])
                cb6.append(ct_)
            CA_t, CB_t, CMAX_t, CLST_t, CS_t, CS2_t = cb6

            # ---- fast path: sample-0 indices so gathers(0) start early ----
            idx0_ts = [scr.tile([128, 64], I16, name=f"idx0s{s}",
                                tag=f"idx0s{s}") for s in range(4)]
            NF = 64

            def f_t(tag, dt=F32):
                return itp.tile([128, 2, 4, NF], dt, name=tag, tag=tag)

            cF = [ct_[:].rearrange("p (s w) -> p s w", s=4)[:, :, 0:NF]
                  .unsqueeze(1).broadcast_to([128, 2, 4, NF])
                  for ct_ in (CA_t, CB_t, CMAX_t, CLST_t)]
            csF = CS_t[:].rearrange("p (s w) -> p s w", s=4)[:, :, 0:NF]
            cs2F = CS2_t[:].rearrange("p (s w) -> p s w", s=4)[:, :, 0:NF]
            rz0 = itp.tile([128, NF], F32, name="rz0", tag="rz0")
            nc.vector.reciprocal(rz0[:], coords[:, 2, 0, :])
            q0t = itp.tile([128, 2, NF], F32, name="q0t", tag="q0t")
            for a, crow in ((0, 1), (1, 0)):
                num = coords[:, crow, 0, :]
                qv = q0t[:, a, :]
                nc.vector.tensor_tensor(qv, num, rz0[:], op=mult)
                t1 = itp.tile([128, NF], F32, name="f_nt1", tag="f_nt1")
                nc.vector.tensor_tensor(t1[:], qv, coords[:, 2, 0, :], op=mult)
                nc.vector.tensor_tensor(t1[:], num, t1[:], op=subtract)
                nc.vector.tensor_tensor(t1[:], t1[:], rz0[:], op=mult)
                nc.vector.tensor_tensor(qv, qv, t1[:], op=add)
            nc.vector.tensor_scalar_mul(q0t[:, 1, :], q0t[:, 1, :], -1.0)
            FT = f_t("FT")
            FU = f_t("FU")
            FV = f_t("FV")
            Fxi = f_t("Fxi", I32)
            for a in range(2):
                for s in range(4):
                    nc.vector.tensor_copy(FT[:, a, s, :], q0t[:, a, :])
            nc.vector.tensor_tensor(FT[:], FT[:], cF[0], op=mult)
            nc.vector.tensor_tensor(FT[:], FT[:], cF[1], op=add)
            FTf = FT.rearrange("p a s w -> p (a s w)")
            FUf = FU.rearrange("p a s w -> p (a s w)")
            FVf = FV.rearrange("p a s w -> p (a s w)")
            Fxif = Fxi.rearrange("p a s w -> p (a s w)")
            nc.vector.tensor_scalar_max(FTf, FTf, 0.0)
            nc.vector.tensor_tensor(FT[:], FT[:], cF[2], op=amin)
            nc.vector.tensor_copy(Fxif, FTf)
            nc.vector.tensor_copy(FUf, Fxif)
            nc.vector.tensor_tensor(FVf, FUf, FTf, op=is_gt)
            nc.vector.tensor_tensor(FUf, FUf, FVf, op=subtract)
            nc.vector.tensor_tensor(FVf, FTf, FUf, op=is_gt)
            nc.vector.tensor_tensor(FTf, FUf, FVf, op=add)
            nc.vector.tensor_tensor(FT[:], FT[:], cF[3], op=amin)
            nc.vector.tensor_tensor(FTf, FTf, FUf, op=subtract)
            nc.vector.tensor_tensor(FV[:, 0], FT[:, 0], FT[:, 1], op=mult)
            nc.vector.tensor_tensor(FT[:, 0], FU[:, 0], csF, op=mult)
            nc.vector.tensor_tensor(FT[:, 0], FT[:, 0], FU[:, 1], op=add)
            nc.vector.tensor_tensor(FT[:, 0], FT[:, 0], cs2F, op=subtract)
            nc.vector.tensor_tensor(FT[:, 0], FT[:, 0], FV[:, 0], op=mult)
            nc.vector.tensor_tensor(FT[:, 0], FT[:, 0], cs2F, op=add)
            for s in (1, 2, 3, 0):
                nc.vector.tensor_copy(idx0_ts[s][:], FT[:, 0, s, :])

            def i_t(tag, wide=True, dt=F32):
                shape = [128, 4, NW] if wide else [128, NW]
                return itp.tile(shape, dt, name=tag, tag=tag)

            rz = i_t("rz", False)
            nc.vector.reciprocal(rz[:], zz)
            qh = i_t("qh", False)
            qw = i_t("qw", False)
            for qt, num in ((qh, yy), (qw, x0)):
                nc.vector.tensor_tensor(qt[:], num, rz[:], op=mult)  # q0
                # one Newton step: q = q0 + rz*(n - q0*z)
                t1 = i_t("nt1", False)
                nc.vector.tensor_tensor(t1[:], qt[:], zz, op=mult)
                nc.vector.tensor_tensor(t1[:], num, t1[:], op=subtract)
                nc.vector.tensor_tensor(t1[:], t1[:], rz[:], op=mult)
                nc.vector.tensor_tensor(qt[:], qt[:], t1[:], op=add)
            nc.vector.tensor_scalar_mul(qw[:], qw[:], -1.0)  # w uses -q

            xi = itp.tile([128, 2, 4, NW], I32, name="xi", tag="xi")
            V = itp.tile([128, 2, 4, NW], F32, name="V", tag="V")
            T = itp.tile([128, 2, 4, NW], F32, name="T", tag="T")
            U = itp.tile([128, 2, 4, NW], F32, name="U", tag="U")
            Tf = T.rearrange("p a s w -> p (a s w)")
            Uf = U.rearrange("p a s w -> p (a s w)")
            Vf = V.rearrange("p a s w -> p (a s w)")
            xif = xi.rearrange("p a s w -> p (a s w)")
            CAb = CA_t[:].unsqueeze(1).broadcast_to([128, 2, 1024])
            CBb = CB_t[:].unsqueeze(1).broadcast_to([128, 2, 1024])
            CMb = CMAX_t[:].unsqueeze(1).broadcast_to([128, 2, 1024])
            CLb = CLST_t[:].unsqueeze(1).broadcast_to([128, 2, 1024])
            for a, qt in ((0, qh), (1, qw)):
                for s in range(4):
                    nc.vector.tensor_copy(T[:, a, s, :], qt[:])
            nc.vector.tensor_tensor(T[:], T[:], CAb, op=mult)
            nc.vector.tensor_tensor(T[:], T[:], CBb, op=add)
            nc.vector.tensor_scalar_max(Tf, Tf, 0.0)
            nc.vector.tensor_tensor(T[:], T[:], CMb, op=amin)        # xf
            nc.vector.tensor_copy(xif, Tf)                           # floor
            nc.vector.tensor_copy(Uf, xif)                           # xq
            nc.vector.tensor_tensor(Vf, Uf, Tf, op=is_gt)
            nc.vector.tensor_tensor(Uf, Uf, Vf, op=subtract)         # fl
            nc.vector.tensor_tensor(Vf, Tf, Uf, op=is_gt)            # hasfrac
            nc.vector.tensor_tensor(Tf, Uf, Vf, op=add)
            nc.vector.tensor_tensor(T[:], T[:], CLb, op=amin)        # x2
            nc.vector.tensor_tensor(Tf, Tf, Uf, op=subtract)         # mx
            fl0 = U[:, 0].rearrange("p s w -> p (s w)")
            fl1 = U[:, 1].rearrange("p s w -> p (s w)")
            mx0 = T[:, 0].rearrange("p s w -> p (s w)")
            mx1 = T[:, 1].rearrange("p s w -> p (s w)")
            Vc = V[:, 0].rearrange("p s w -> p (s w)")
            nc.vector.tensor_tensor(Vc, mx0, mx1, op=mult)           # mask
            nc.vector.tensor_tensor(mx0, fl0, CS_t[:], op=mult)
            nc.vector.tensor_tensor(mx0, mx0, fl1, op=add)           # idx
            nc.vector.tensor_tensor(mx0, mx0, CS2_t[:], op=subtract)
            nc.vector.tensor_tensor(mx0, mx0, Vc, op=mult)
            nc.vector.tensor_tensor(mx0, mx0, CS2_t[:], op=add)
            for s in (1, 2, 3, 0):
                nc.vector.tensor_copy(
                    idx_ts[s].rearrange("p b i -> p (b i)"),
                    T[:, 0, s, :].rearrange("p b i -> p (b i)") if False
                    else mx0[:, s * NW:(s + 1) * NW])
        idx_tiles = idx_ts
        idx0_tiles = idx0_ts

        # ---- gathers -------------------------------------------------------
        g_tiles = [None] * BPC

        def emit_gathers(b):
            tiles = [None] * 4
            for s in (1, 2, 3, 0):   # fc consumes m-chunks in this scale order
                C = GCH[s]
                gt_ = gp.tile([128, 8, C], BF16, name=f"g{s}_{b}", tag=f"g{s}")
                nc.gpsimd.dma_gather(
                    out_ap=gt_[:],
                    in_ap=its[s][b],
                    idxs_ap=(idx0_tiles[s][:] if b == 0
                             else idx_tiles[s][:, b, :]),
                    num_idxs=N,
                    num_idxs_reg=N,
                    elem_size=C,
                    queue_num=0,
                )
                tiles[s] = gt_
            g_tiles[b] = tiles

        emit_gathers(0)
        emit_gathers(1)

        # ---- persistent weights -------------------------------------------
        fcw_t = wp.tile([128, 8, 1024], BF16, name="fcw")
        nc.sync.dma_start(out=fcw_t[:], in_=fcwT[:].rearrange("(k p) o -> p k o", p=128))
        c1wa = wp.tile([128, 7, 1024], BF16, name="c1wa")
        nc.sync.dma_start(out=c1wa[:],
                          in_=c1wT[0:896].rearrange("(k p) o -> p k o", p=128))
        c1wb = wp.tile([70, 1024], BF16, name="c1wb")
        nc.sync.dma_start(out=c1wb[:], in_=c1wT[896:966])
        w32_t = wp.tile([128, 8, 64], BF16, name="w32t")
        nc.sync.dma_start(out=w32_t[:], in_=w32T[:].rearrange("(k p) m -> p k m", p=128))
        wf_t = wp.tile([64, 6], BF16, name="wft")
        nc.sync.dma_start(out=wf_t[:], in_=wfT[:])
        b32_t = wp.tile([64, 1], F32, name="b32t")
        nc.sync.dma_start(out=b32_t[:], in_=b32v[:].unsqueeze(1))
        bf_t = wp.tile([6, 1], F32, name="bft")
        nc.sync.dma_start(out=bf_t[:], in_=bfv[:].unsqueeze(1))
        c1b_t = wp.tile([128, 8], F32, name="c1bt")
        nc.sync.dma_start(out=c1b_t[:], in_=c1b[:].rearrange("(m p) -> p m", p=128))
        codes_t = wp.tile([128, 16, BPC], F32, name="codest")
        nc.sync.dma_start(out=codes_t[:],
                          in_=codesT[:].rearrange("(k p) b -> p k b", p=128))


        # ---- CT = c1_cg @ [code; global_code] + c1_b  (all samples) -------
        # PSUM holds only one k-step (per-matmul start/stop); accumulate in
        # SBUF to avoid 8 interleaved accumulation groups in one bank.
        ct_sb = wp.tile([128, 8, BPC], F32, name="ctsb")
        nc.vector.memset(ct_sb[:], 0.0)
        for k in range(16):
            cg_t = cgp.tile([128, 1024], F32, name=f"cg{k}", tag="cg")
            nc.sync.dma_start(out=cg_t[:], in_=cgT[128 * k:128 * (k + 1)])
            ctp = psE.tile([128, 8, BPC], F32, name="ctp", tag="ctp")
            for m in range(8):
                nc.tensor.matmul(
                    ctp[:, m, :], (cg_t[:, 128 * m:128 * (m + 1)]),
                    (codes_t[:, k, :]), start=True, stop=True,
                )
            acc_v = ct_sb.rearrange("p m b -> p (m b)")
            nc.vector.tensor_tensor(acc_v, acc_v,
                                    ctp.rearrange("p m b -> p (m b)"), op=add)
        for m in range(8):
            nc.vector.tensor_scalar(out=ct_sb[:, m, :], in0=ct_sb[:, m, :],
                                    scalar1=c1b_t[:, m:m + 1], scalar2=None,
                                    op0=add)

        # ---- per-sample pipeline ------------------------------------------
        for b in range(BPC):
            g0, g1, g2, g3 = g_tiles[b]
            gmap = {0: g0, 1: g1, 2: g2, 3: g3}
            for oc in range(2):
                osl = slice(oc * NH, (oc + 1) * NH)
                # fc: Y[c, o] = sum_n GT[n, c] fcwT[n, o]
                ye = [yep.tile([128, NH], BF16, name=f"ye{k}", tag=f"ye{k}")
                      for k in range(7)]
                ye7 = yep.tile([70, NH], BF16, name="ye7", tag="ye7")
                for m, (s, c0, cw) in enumerate(MCHUNKS):
                    yp = psA.tile([cw, NH], F32, name="yp", tag="yp")
                    for k in range(8):
                        nc.tensor.matmul(
                            yp[:], (gmap[s][:, k, c0:c0 + cw]),
                            (fcw_t[:, k, osl]),
                            start=(k == 0), stop=(k == 7),
                        )
                    dst = ye[m][:] if m < 7 else ye7[0:64, :]
                    nc.vector.tensor_copy(dst, yp[:])
                nc.sync.dma_start(out=ye7[64:67, :], in_=lvl_bf[b][:, osl])
                nc.sync.dma_start(out=ye7[67:69, :], in_=grid2[:, osl])
                nc.sync.dma_start(out=ye7[69:70, :], in_=fcb[osl].unsqueeze(0))

                # c1
                f1 = [f1sb.tile([128, NH], BF16, name=f"f1_{m}", tag=f"f1_{m}")
                      for m in range(8)]
                for m in range(8):
                    fp1 = psB.tile([128, NH], F32, name="f1p", tag="f1p")
                    for k in range(8):
                        if k < 7:
                            nc.tensor.matmul(
                                fp1[:], (c1wa[:, k, 128 * m:128 * (m + 1)]),
                                (ye[k][:]), start=(k == 0), stop=False)
                        else:
                            nc.tensor.matmul(
                                fp1[:], (c1wb[:, 128 * m:128 * (m + 1)]),
                                (ye7[:]), start=False, stop=True)
                    nc.vector.tensor_scalar(out=f1[m][:], in0=fp1[:],
                                            scalar1=ct_sb[:, m, b:b + 1],
                                            scalar2=None, op0=add)

                # W32 + relu
                rp = psC.tile([64, NH], F32, name="rp", tag="rp")
                for k in range(8):
                    nc.tensor.matmul(rp[:], (w32_t[:, k, :]), (f1[k][:]),
                                     start=(k == 0), stop=(k == 7))
                f1r = outp.tile([64, NH], BF16, name="f1r", tag="f1r")
                nc.scalar.activation(f1r[:], rp[:],
                                     mybir.ActivationFunctionType.Relu,
                                     bias=b32_t[:], scale=1.0)

                # WF
                fpp = psD.tile([6, NH], F32, name="fpp", tag="fpp")
                nc.tensor.matmul(fpp[:], (wf_t[:]), (f1r[:]))
                feat_sb = outp.tile([6, NH], F32, name="featsb", tag="featsb")
                nc.vector.tensor_scalar(out=feat_sb[:], in0=fpp[:],
                                        scalar1=bf_t[:], scalar2=None, op0=add)
                nc.sync.dma_start(out=feat[b][:, osl], in_=feat_sb[:])

            if b + 2 < BPC:
                emit_gathers(b + 2)

    nc.compile()
    return nc


def fold_weights(inp):
    f64 = np.float64
    g = lambda k: np.asarray(inp[k], f64)
    w2d1s = g('w2d1').sum(-1)
    W21 = g('w2d2') @ w2d1s
    b21 = g('w2d2') @ g('b2d1') + g('b2d2')
    BD3 = np.zeros((64, 128), f64)
    for u in range(2):
        BD3[np.arange(32) * 2 + u, u * 64:(u + 1) * 64] = g('w2d3')
    b3x = np.zeros(64, f64)
    b3x[0::2] = g('b2d3'); b3x[1::2] = g('b2d3')
    W321 = BD3 @ W21
    b321 = BD3 @ b21 + b3x
    W65 = g('c6_w') @ g('c5_w')
    b65 = g('c6_w') @ g('c5_b') + g('c6_b')
    W64 = W65 @ g('c4_w')
    b64 = W65 @ g('c4_b') + b65
    WF = W64 @ (np.eye(64) + W321)
    bF = W64 @ b321 + b64
    W32 = g('c3_w') @ g('c2_w')
    b32 = g('c3_w') @ g('c2_b') + g('c3_b')
    return (WF.astype(np.float32), bF.astype(np.float32),
            W32.astype(np.float32), b32.astype(np.float32))


def prep_in_maps(inputs):
    """Full inputs -> per-core in_maps for run_bass_kernel_spmd."""
    f32 = np.float32
    WF, bF, W32, b32 = fold_weights(inputs)
    c1_w = np.ascontiguousarray(np.asarray(inputs['c1_w'], f32))
    ci = c1_w[:, 2053:3013]
    # kernel channel order: img1, img2, img3, img0, point, grid, fc_b-row
    c1wT = np.concatenate([
        ci[:, 64:192].T, ci[:, 192:448].T, ci[:, 448:960].T, ci[:, 0:64].T,
        c1_w[:, 2:5].T, c1_w[:, 0:2].T, ci.sum(axis=1)[None, :],
    ], axis=0).astype(BF)                                    # [966, 1024]
    cgT = np.ascontiguousarray(c1_w[:, 5:2053].T)            # [2048, 1024]
    fcwT = np.ascontiguousarray(np.asarray(inputs['fc_w'], f32).T.astype(BF))
    w32T = np.ascontiguousarray(W32.T.astype(BF))
    wfT = np.ascontiguousarray(WF.T.astype(BF))
    grid2 = np.zeros((2, N), BF)
    grid2[0, 0::2] = BF(-0.2)
    grid2[0, 1::2] = BF(0.2)
    grid2[1, :] = BF(-0.2)

    ctab = np.zeros((6, 1024), f32)
    for s, S in enumerate(IMG_SIZES):
        sl = slice(s * 256, (s + 1) * 256)
        ctab[0, sl] = f32(248.0) * f32(S / 224.0)
        ctab[1, sl] = f32(111.5) * f32(S / 224.0)
        ctab[2, sl] = f32(223.0) * f32(S / 224.0)
        ctab[3, sl] = f32(S - 1)
        ctab[4, sl] = f32(S)
        ctab[5, sl] = f32(S * S)

    codes = np.concatenate([np.asarray(inputs['code'], f32),
                            np.asarray(inputs['global_code'], f32)],
                           axis=1)                           # [B, 2048]
    level0 = np.asarray(inputs['level0'], f32)

    imgsT = []
    for i, S in enumerate(IMG_SIZES):
        img = np.asarray(inputs[f'img{i}'], f32)
        C = img.shape[1]
        t = img.reshape(B, C, S * S).transpose(0, 2, 1).astype(BF)
        full = np.zeros((B, S * S + 1, GCH[i]), BF)          # zero row + pad
        full[:, :S * S, :C] = t
        imgsT.append(full)

    shared = dict(fcwT=fcwT, c1wT=c1wT, cgT=cgT,
                  c1b=np.ascontiguousarray(np.asarray(inputs['c1_b'], f32)),
                  w32T=w32T, b32v=b32, wfT=wfT, bfv=bF,
                  grid2=grid2, ctab=ctab,
                  fcb=np.ascontiguousarray(
                      np.asarray(inputs['fc_b'], f32).astype(BF)))
    in_maps = []
    for c in range(NCORES):
        sl = slice(c * BPC, (c + 1) * BPC)
        m = dict(shared)
        m['lvl'] = np.ascontiguousarray(level0[sl])
        m['lvl_bf'] = np.ascontiguousarray(level0[sl].astype(BF))
        m['codesT'] = np.ascontiguousarray(codes[sl].T)
        for i in range(4):
            m[f'it{i}'] = imgsT[i][sl]
        in_maps.append(m)
    return in_maps


def assemble(results):
    out = np.zeros((B, 2 * N, 3), np.float32)
    for c in range(NCORES):
        featc = results[c]['feat']                   # [BPC, 6, N]
        for j in range(3):
            out[c * BPC:(c + 1) * BPC, :N, j] = featc[:, 2 * j, :]
            out[c * BPC:(c + 1) * BPC, N:, j] = featc[:, 2 * j + 1, :]
    return out


_NC_CACHE = None


def get_nc():
    global _NC_CACHE
    if _NC_CACHE is None:
        _NC_CACHE = build_nc()
    return _NC_CACHE


def kernel(**inputs):
    global LAST_RESULTS
    nc = get_nc()
    in_maps = prep_in_maps(inputs)
    res = run_bass_kernel_spmd(nc, in_maps, core_ids=list(range(NCORES)),
                               trace=TRACE)
    LAST_RESULTS = res
    return assemble(res.results)
